# revision 3
# baseline (speedup 1.0000x reference)
"""VQ codebook soft-assignment encoding kernel for 8 trn2 NeuronCores. v2.

Math (per batch b):
  Xf = X[b].reshape(D, N).T                        # [N, D], N = H*W
  logit[n,k] = scale[k] * (||x_n||^2 - 2<x_n,c_k> + ||c_k||^2)
  A = softmax(logit, axis=k)
  E[b,k,:] = sum_n A[n,k] * (x_n - c_k)            # [K, D]

Sharding: data-parallel over B (4 batches per core), codewords/scale replicated.

v2 changes vs baseline: no DMA-xbar transposes at all (they serialized the
entire DMA timeline against the HBM loads). X^T is produced on the PE in
transpose-mode and evacuated PSUM->SBUF on ACT/DVE; logits and E matmuls are
4-way column-tiled (M=K=32 -> 4 concurrent col-groups); exp runs on 128
partitions; the 4 E partial accumulators are folded by one [128,32]^T matmul.
"""
import numpy as np
from contextlib import ExitStack

import concourse.bass as bass
import concourse.mybir as mybir
import concourse.tile as tile
from concourse.tile import ScopedClock
from concourse.bass_utils import run_bass_kernel_spmd

dt = mybir.dt

B, D, K, H, W = 32, 256, 32, 96, 96
N = H * W                 # 9216
NCORES = 8
BPC = B // NCORES         # 4 batches per core
TN = 512                  # n-tile for logits
NT = N // TN              # 18 tiles -> 4 full groups of 4 + 1 half group of 2
NCHUNK = N // 128         # 72 chunks for E-matmul / transposes
XTW = 260                 # per-chunk column stride in XT16 (256 d + ones + pad)
WCH = 8                   # chunks per X-transpose evacuation wave
NWAVE = NCHUNK // WCH     # 9 waves
EVAC_ACT = 9              # leading waves evacuated by ACT (rest DVE), 0..NWAVE
XSQ_DVE = 2048            # x0^2 columns computed on DVE (rest ACT; x1^2 all DVE)


def _patch_tile_drain():
    """This toolchain's walrus allows only one sync-wait per instruction.
    Split the tail drain's waits across chained drains."""
    if getattr(tile.TileContext, "_drain_patched", False):
        return

    def _drain_and_barrier_split(self, tick_clock, wait_clock):
        nc = self.nc
        drain_inst = nc.sync.drain()
        wait_clock.add_sem_waits(
            drain_inst.ins, ScopedClock({None: tick_clock.global_clock})
        )
        si = drain_inst.ins.sync_info
        if si is not None and si.on_wait and len(si.on_wait) > 1:
            extra = list(si.on_wait[1:])
            del si.on_wait[1:]
            for w in extra:
                d = nc.sync.drain()
                dsi = d.ins.sync_info
                if dsi is None:
                    d.ins.sync_info = mybir.SyncInfo(on_wait=[w], on_update=[])
                else:
                    dsi.on_wait.append(w)
        nc.all_engine_barrier()
        assert self.sems is not None
        popped = nc._tile_sem_poison_stack.pop()
        assert popped is self._sem_poison
        nc.clear_and_free_semaphores(list(self.sems.allocated().values()))
        nc.all_engine_barrier()

    tile.TileContext._drain_and_barrier = _drain_and_barrier_split
    tile.TileContext._drain_patched = True


def _split_multi_waits(nc):
    """Hoist extra sem-waits onto standalone event-sem instructions."""
    n_split = 0
    for f in nc.m.functions:
        for bb in f.blocks:
            new_list = []
            for inst in bb.instructions:
                si = inst.sync_info
                if si is not None and si.on_wait is not None and len(si.on_wait) > 1:
                    extra = list(si.on_wait[:-1])
                    keep = [si.on_wait[-1]]
                    for w in extra:
                        ev = mybir.InstEventSemaphore(
                            name=f"{inst.name}-wsplit{n_split}",
                            ins=[], outs=[],
                            sync_info=mybir.SyncInfo(on_wait=[w], on_update=[]),
                        )
                        ev.engine = inst.engine
                        nc.register_instruction(ev)
                        new_list.append(ev)
                        n_split += 1
                    del si.on_wait[:]
                    si.on_wait.extend(keep)
                new_list.append(inst)
            bb.instructions[:] = new_list
    return n_split


def _build_module(bpc=BPC, sim_safe=False):
    _patch_tile_drain()
    nc = bass.Bass()
    xin = nc.declare_dram_parameter("xin", [bpc, D, N], dt.float32, isOutput=False)
    cw = nc.declare_dram_parameter("cw", [K, D], dt.float32, isOutput=False)
    s_col = nc.declare_dram_parameter("s_col", [K, 1], dt.float32, isOutput=False)
    s_row = nc.declare_dram_parameter("s_row", [1, K], dt.float32, isOutput=False)
    id128 = nc.declare_dram_parameter("id128", [128, 128], dt.float16, isOutput=False)
    fold4 = nc.declare_dram_parameter("fold4", [128, K], dt.float16, isOutput=False)
    eout = nc.declare_dram_parameter("eout", [bpc, K, D], dt.float32, isOutput=True)

    f16, f32 = dt.float16, dt.float32
    AX = mybir.AxisListType.X
    EXP = mybir.ActivationFunctionType.Exp
    SQ = mybir.ActivationFunctionType.Square
    CP = mybir.ActivationFunctionType.Copy

    with tile.TileContext(nc) as tc:
        with ExitStack() as ctx:
            singles = ctx.enter_context(tc.tile_pool(name="singles", bufs=1))
            psl = ctx.enter_context(tc.tile_pool(name="psl", bufs=2, space="PSUM"))

            # ---- one-time prep from codewords/scale ----
            cw_sb = singles.tile([K, D], f32)
            nc.sync.dma_start(cw_sb[:], cw[:])
            scol_sb = singles.tile([K, 1], f32)
            nc.sync.dma_start(scol_sb[:], s_col[:])
            srow_sb = singles.tile([1, K], f32)
            nc.sync.dma_start(srow_sb[:], s_row[:])
            id16 = singles.tile([128, 128], f16)
            nc.sync.dma_start(id16[:], id128[:])
            fold16 = singles.tile([128, K], f16)
            nc.sync.dma_start(fold16[:], fold4[:])

            # G16 [128, 2, K]: G[p, c, k] = -2 s_k c[k, c*128+p]
            w1 = singles.tile([K, D], f32)
            nc.vector.tensor_scalar_mul(w1[:], cw_sb[:], scol_sb[:])
            w2 = singles.tile([K, D], f32)
            nc.vector.tensor_scalar_mul(w2[:], w1[:], -2.0)
            w16 = singles.tile([K, D], f16)
            nc.vector.tensor_copy(w16[:], w2[:])
            g16 = singles.tile([128, 2 * K], f16)
            g3t = g16[:].rearrange("p (c k) -> p c k", k=K)
            wv = w16[:].rearrange("k (c j w) -> k c j w", j=4, w=32)
            for j in range(4):
                if sim_safe:
                    for c in range(2):
                        nc.vector.transpose(g3t[32 * j:32 * (j + 1), c, :],
                                            wv[:, c, j, :])
                else:
                    nc.vector.transpose(g3t[32 * j:32 * (j + 1), :, :],
                                        wv[:, :, j, :])

            # S16 [128, K]: every row = s_k (fp16)
            ones_row16 = singles.tile([1, 128], f16)
            nc.vector.memset(ones_row16[:], 1.0)
            ones_col16 = singles.tile([128, 1], f16)
            nc.vector.memset(ones_col16[:], 1.0)
            srow16 = singles.tile([1, K], f16)
            nc.vector.tensor_copy(srow16[:], srow_sb[:])
            ps_s = psl.tile([128, TN], f32, tag="psl")
            nc.tensor.matmul(ps_s[:, 0:K], ones_row16[:], srow16[:], start=True,
                             stop=True, skip_group_check=True)
            s16 = singles.tile([128, K], f16)
            nc.vector.tensor_copy(s16[:], ps_s[:, 0:K])

            # bias128 [128, 1]: bias[32j + k] = s_k * ||c_k||^2 (4 stripes)
            csq = singles.tile([K, D], f32)
            nc.vector.tensor_mul(csq[:], cw_sb[:], cw_sb[:])
            sqc = singles.tile([K, 1], f32)
            nc.vector.reduce_sum(
                sqc[:].rearrange("k (o p) -> k o p", o=1),
                csq[:].rearrange("k (o d) -> k o d", o=1), axis=AX)
            bias = singles.tile([K, 1], f32)
            nc.vector.tensor_mul(bias[:], sqc[:], scol_sb[:])
            bias128 = singles.tile([128, 1], f32)
            for j in range(4):
                nc.vector.tensor_copy(bias128[32 * j:32 * (j + 1), :], bias[:])

            # ---- per-batch pools ----
            xpool = ctx.enter_context(tc.tile_pool(name="x16", bufs=2))
            sqpool = ctx.enter_context(tc.tile_pool(name="xsq", bufs=1))
            upool = ctx.enter_context(tc.tile_pool(name="u16", bufs=2))
            utpool = ctx.enter_context(tc.tile_pool(name="ut16", bufs=2))
            npool = ctx.enter_context(tc.tile_pool(name="nrm", bufs=4))
            atpool = ctx.enter_context(tc.tile_pool(name="at16", bufs=2))
            xtpool = ctx.enter_context(tc.tile_pool(name="xt16", bufs=1))
            fpool = ctx.enter_context(tc.tile_pool(name="f16", bufs=2))
            opool = ctx.enter_context(tc.tile_pool(name="out", bufs=2))
            psxt = ctx.enter_context(tc.tile_pool(name="psxt", bufs=2, space="PSUM"))
            pse = ctx.enter_context(tc.tile_pool(name="pse", bufs=2, space="PSUM"))

            g16v = g16[:].rearrange("p (c k) -> p c k", k=K)

            def batch_tail(pe, b):
                # fold 4 partials + E = psF[:, :256] - S_k c; emitted at the
                # TOP of the next slot so no engine queue blocks on E(b)
                ef16 = fpool.tile([128, 257], f16, tag="f")
                nc.scalar.activation(ef16[:], pe[:, 0:257], CP)
                psf = psl.tile([128, TN], f32, tag="psl")
                nc.tensor.matmul(psf[0:K, 0:257], fold16[:], ef16[:],
                                 start=True, stop=True, skip_group_check=True)
                cs = opool.tile([K, D], f32, tag="cs")
                nc.vector.tensor_scalar_mul(cs[:], cw_sb[:], psf[0:K, 256:257])
                ef = opool.tile([K, D], f32, tag="ef")
                nc.vector.tensor_sub(ef[:], psf[0:K, 0:256], cs[:])
                nc.sync.dma_start(eout[b], ef[:])

            prev = None
            for b in range(bpc):
                x0 = xpool.tile([128, N], f16, tag="x0")
                x1 = xpool.tile([128, N], f16, tag="x1")
                if b == 0:
                    # batch 0 is the cold prologue (nothing overlaps its
                    # load): split in halves so logits groups 0-1 start
                    # ~13us earlier. Later batches keep whole loads -- their
                    # steady-state schedule is a measured local optimum.
                    hn = 9 * TN
                    nc.gpsimd.dma_start(x0[:, 0:hn], xin[b, 0:128, 0:hn])
                    nc.gpsimd.dma_start(x1[:, 0:hn], xin[b, 128:256, 0:hn])
                    nc.gpsimd.dma_start(x0[:, hn:N], xin[b, 0:128, hn:N])
                    nc.gpsimd.dma_start(x1[:, hn:N], xin[b, 128:256, hn:N])
                else:
                    nc.gpsimd.dma_start(x0[:], xin[b, 0:128, :])
                    nc.gpsimd.dma_start(x1[:], xin[b, 128:256, :])
                if prev is not None:
                    batch_tail(*prev)

                # U16 [128, 5*512]: group g cols [512g, 512g+512);
                # partition 32j+k holds n-tile t=4g+j (g=4: j in {0,1}).
                u16 = upool.tile([128, 5 * TN], f16, tag="u")

                # x^2 up front in big calls: xsq[:, 0:N] = x0^2 (DVE head,
                # ACT tail), xsq[:, N:2N] = x1^2 (DVE)
                xsq = sqpool.tile([128, 2 * N], f16, tag="xsq")
                nc.vector.tensor_mul(xsq[:, 0:XSQ_DVE], x0[:, 0:XSQ_DVE],
                                     x0[:, 0:XSQ_DVE])
                if b == 0:
                    hn = 9 * TN
                    nc.scalar.activation(xsq[:, XSQ_DVE:hn],
                                         x0[:, XSQ_DVE:hn], SQ)
                    nc.scalar.activation(xsq[:, hn:N], x0[:, hn:N], SQ)
                    nc.vector.tensor_mul(xsq[:, N:N + hn], x1[:, 0:hn],
                                         x1[:, 0:hn])
                    nc.vector.tensor_mul(xsq[:, N + hn:2 * N], x1[:, hn:N],
                                         x1[:, hn:N])
                else:
                    nc.scalar.activation(xsq[:, XSQ_DVE:N], x0[:, XSQ_DVE:N],
                                         SQ)
                    nc.vector.tensor_mul(xsq[:, N:2 * N], x1[:], x1[:])

                # ---- logits (col-tiled) + exp, per group of 4 n-tiles ----
                for g in range(5):
                    nj = 4 if g < 4 else 2
                    pl = psl.tile([128, TN], f32, tag="psl")
                    for j in range(nj):
                        t = 4 * g + j
                        ts = bass.ts(t, TN)
                        ps_j = pl[32 * j:32 * (j + 1), :]
                        tp = (0, 32 * j)
                        nc.tensor.matmul(ps_j, g16v[:, 0, :], x0[:, ts],
                                         start=True, stop=False,
                                         skip_group_check=True, tile_position=tp)
                        nc.tensor.matmul(ps_j, g16v[:, 1, :], x1[:, ts],
                                         start=False, stop=False,
                                         skip_group_check=True, tile_position=tp)
                        nc.tensor.matmul(ps_j, s16[:], xsq[:, ts],
                                         start=False, stop=False,
                                         skip_group_check=True, tile_position=tp)
                        nc.tensor.matmul(ps_j, s16[:], xsq[:, N + 512 * t:
                                                           N + 512 * (t + 1)],
                                         start=False, stop=True,
                                         skip_group_check=True, tile_position=tp)
                    npart = 32 * nj
                    nc.scalar.activation(u16[0:npart, bass.ts(g, TN)],
                                         pl[0:npart, :], EXP,
                                         bias=bias128[0:npart, :], scale=1.0)

                # ---- U^T on DVE: UT16 [128, 72*32], col = 32*tt + k ----
                # tt = 16g + 4j + q (g<4), tt = 64 + 4j + q (g=4, j<2)
                ut16 = utpool.tile([128, NCHUNK * K], f16, tag="ut")
                utf = ut16[:, 0:2048].rearrange("p (g j q k) -> p g j q k",
                                                g=4, j=4, q=4)
                uf = u16[:, 0:2048].rearrange("p (g q a w) -> p g q a w",
                                              g=4, q=4, a=4)
                for j in range(4):
                    for a in range(4):
                        if sim_safe:
                            for g in range(4):
                                for q in range(4):
                                    nc.vector.transpose(
                                        utf[32 * a:32 * (a + 1), g, j, q, :],
                                        uf[32 * j:32 * (j + 1), g, q, a, :])
                        else:
                            nc.vector.transpose(
                                utf[32 * a:32 * (a + 1), :, j, :, :],
                                uf[32 * j:32 * (j + 1), :, :, a, :])
                uth = ut16[:, 2048:2304].rearrange("p (j q k) -> p j q k", j=2, q=4)
                uh = u16[:, 2048:2560].rearrange("p (q a w) -> p q a w", q=4, a=4)
                for j in range(2):
                    for a in range(4):
                        if sim_safe:
                            for q in range(4):
                                nc.vector.transpose(
                                    uth[32 * a:32 * (a + 1), j, q, :],
                                    uh[32 * j:32 * (j + 1), q, a, :])
                        else:
                            nc.vector.transpose(
                                uth[32 * a:32 * (a + 1), j, :, :],
                                uh[32 * j:32 * (j + 1), :, a, :])

                # den, recip, A^T = U^T * (1/den): emitted in halves,
                # interleaved with the E-matmul halves further below
                at16 = atpool.tile([128, NCHUNK * K], f16, tag="at")
                den = npool.tile([128, NCHUNK], f32, tag="den")
                rec = npool.tile([128, NCHUNK], f32, tag="rec")
                rec16 = npool.tile([128, NCHUNK], f16, tag="rec16")
                HC = NCHUNK // 2

                def norm_half(h):
                    hs = slice(h * HC, (h + 1) * HC)
                    hks = slice(h * HC * K, (h + 1) * HC * K)
                    nc.vector.reduce_sum(
                        den[:, hs].rearrange("p (t o) -> p t o", o=1),
                        ut16[:, hks].rearrange("p (t k) -> p t k", k=K), axis=AX)
                    nc.vector.reciprocal(rec[:, hs], den[:, hs])
                    nc.vector.tensor_copy(rec16[:, hs], rec[:, hs])
                    recb = rec16[:, hs].rearrange("p (t o) -> p t o", o=1)
                    recb = recb.broadcast_to((128, HC, K))
                    nc.vector.tensor_mul(
                        at16[:, hks].rearrange("p (t k) -> p t k", k=K),
                        ut16[:, hks].rearrange("p (t k) -> p t k", k=K), recb)

                # ---- X^T on PE (transpose mode), evac ACT/DVE ----
                # XT16 [128, 72*260]: chunk t cols [260t, 260t+256) = x[n,:],
                # col 260t+256 = 1.0
                xt16 = xtpool.tile([128, NCHUNK * XTW], f16, tag="xt")
                xt3 = xt16[:].rearrange("p (t c) -> p t c", c=XTW)
                nc.vector.memset(xt3[:, :, 256:257], 1.0)
                for w in range(NWAVE):
                    pw = psxt.tile([128, WCH * 256], f16, tag="psxt")
                    for c in range(WCH):
                        t = WCH * w + c
                        ts = bass.ts(t, 128)
                        nc.tensor.transpose(pw[:, 256 * c:256 * c + 128],
                                            x0[:, ts], id16[:])
                        nc.tensor.transpose(pw[:, 256 * c + 128:256 * (c + 1)],
                                            x1[:, ts], id16[:])
                    dst = xt3[:, WCH * w:WCH * (w + 1), 0:256]
                    src = pw[:].rearrange("p (t c) -> p t c", c=256)
                    if w < EVAC_ACT:
                        nc.scalar.activation(dst, src, CP)
                    else:
                        nc.vector.tensor_copy(dst, src)

                # ---- E-matmul: 4 col-tiled partial accumulators, emitted in
                # halves so E chunks 0..35 overlap the second normalize half
                pe = pse.tile([128, TN], f32, tag="pse")
                for h in range(2):
                    norm_half(h)
                    for t in range(h * HC, (h + 1) * HC):
                        j = t % 4
                        nc.tensor.matmul(pe[32 * j:32 * (j + 1), 0:257],
                                         at16[:, bass.ts(t, K)],
                                         xt16[:, XTW * t:XTW * t + 257],
                                         start=(t < 4), stop=(t >= NCHUNK - 4),
                                         skip_group_check=True,
                                         tile_position=(0, 32 * j))

                prev = (pe, b)
            batch_tail(*prev)

    _split_multi_waits(nc)
    return nc


_NC_CACHE = None


def _run(X, codewords, scale, trace=False, tmpdir=None):
    global _NC_CACHE
    if _NC_CACHE is None:
        _NC_CACHE = _build_module()
    nc = _NC_CACHE
    Xr = np.ascontiguousarray(X.reshape(B, D, N), dtype=np.float32)
    cw = np.ascontiguousarray(codewords, dtype=np.float32)
    s = np.asarray(scale, dtype=np.float32).reshape(-1)
    id128 = np.eye(128, dtype=np.float16)
    fold4 = np.zeros((128, K), dtype=np.float16)
    for p in range(128):
        fold4[p, p % K] = 1.0
    in_maps = []
    for c in range(NCORES):
        in_maps.append({
            "xin": Xr[c * BPC:(c + 1) * BPC],
            "cw": cw,
            "s_col": np.ascontiguousarray(s.reshape(K, 1)),
            "s_row": np.ascontiguousarray(s.reshape(1, K)),
            "id128": id128,
            "fold4": fold4,
        })
    kr = run_bass_kernel_spmd(nc, in_maps, list(range(NCORES)),
                              trace=trace, tmpdir=tmpdir)
    out = np.concatenate([r["eout"] for r in kr.results], axis=0)
    return out.astype(np.float32), kr


def kernel(X, codewords, scale):
    out, _ = _run(X, codewords, scale)
    return out
